# revision 49
# baseline (speedup 1.0000x reference)
"""Trainium2 Bass kernel for nn_Bilinear_86328842650062.

Computes out[s,i,j] = sum_{d,e} tensor1[s,i,d] * W[d,e] * tensor0[s,j,e] + bias
for S=4, N=4096, D=64, then tiles to batch 2:  output (2, 4, 4096, 4096) f32.

Strategy (classic 1D row-parallel): shard the i axis (rows of tensor1 /
rows of the output) across 8 NeuronCores, 512 rows each; replicate the
small (D,D) kernel and tensor0.  Per core and per s:
    B[s] = x1_shard[s] @ W            (512x64 @ 64x64, one f32 matmul)
    out_shard[s] = B[s] @ x0[s]^T     (512x64 @ 64x4096, 4x8 PE tiles)
Host-side we pre-transpose x0/x1 so the contraction dim (64) lands on
SBUF partitions, packing two s-slices per 128 partitions for
full-bandwidth DMA.  The batch-2 leading dim is a pure broadcast and is
materialized host-side as a stride-0 view.

The big matmul runs on fp16 operands (full-rate 1 cycle/row on the PE
vs 2 for f32r / 4 for fp32) with a compensated hi/lo split:
    B = Bhi + Blo (fp16 pair, computed on-device from the exact f32 B)
    x0 = Xhi + Xlo (fp16 pair, split on host)
    out ~= Bhi@Xhi + Blo@Xhi + Bhi@Xlo   (PSUM accumulates in f32)
PASSES=3 keeps all three terms (~1e-6 rel error, fp32-class);
PASSES=2 drops Bhi@Xlo (~2e-4); PASSES=1 is plain fp16 (~3e-4).
"""

import os as _os

import numpy as np

S, N, D = 4, 4096, 64
N_CORES = 8
ROWS = N // N_CORES  # 512 output rows per core
BATCH = 2

PASSES = int(_os.environ.get("BASS_PASSES", "1"))
BIG_DT = _os.environ.get("BASS_BIG_DT", "float16")  # "float16" | "float32r"
OUT_DT = _os.environ.get("BASS_OUT_DT", "float16")  # "float32" | "float16"
WARMUP = int(_os.environ.get("BASS_WARMUP", "12"))  # PE warm-up matmuls

_CACHE = {}


def _build(passes, big_dt_name, out_dt_name):
    import concourse.bacc as bacc
    import concourse.tile as tile
    import concourse.mybir as mybir

    dt = mybir.dt
    f32 = dt.float32
    f16 = getattr(dt, big_dt_name)

    nc = bacc.Bacc(
        "TRN2",
        target_bir_lowering=False,
        debug=False,
        enable_asserts=False,
        num_devices=N_CORES,
    )
    # DRAM I/O. x1t is the tensor1 shard transposed to (S, D, ROWS); x0h/x0l
    # are the fp16 hi/lo split of tensor0 transposed to (S, D, N).
    f32r = dt.float32r
    # W and x1t feed the small matmul in f32r (1 instr, ~630ns vs ~2.1us
    # for fp32); the f32r truncation (~1e-4) is buried under the fp16
    # rounding of B that follows anyway.  They arrive pre-packed on the
    # host into ONE partition-major array [128, D + S//2*ROWS] so a single
    # DMA (instead of four, ~2us latency each) unblocks the first matmul:
    # cols [0:D] = W replicated to both halves, then x1t for a=0, a=1.
    wx1_dram = nc.dram_tensor(
        "wx1", [128, D + (S // 2) * ROWS], f32r, kind="ExternalInput"
    ).ap()
    x0h_dram = nc.dram_tensor("x0h", [S, D, N], f16, kind="ExternalInput").ap()
    if passes >= 3:
        x0l_dram = nc.dram_tensor("x0l", [S, D, N], f16, kind="ExternalInput").ap()
    out_dt = getattr(dt, out_dt_name)
    out_dram = nc.dram_tensor("out", [S, ROWS, N], out_dt, kind="ExternalOutput").ap()

    IT = ROWS // 128  # 4 psum row-tiles per s
    JT = N // 512     # 8 psum col-tiles per row-tile

    with tile.TileContext(nc) as tc:
        with (
            tc.tile_pool(name="const", bufs=1) as const_pool,
            tc.tile_pool(name="bt", bufs=2) as bt_pool,
            tc.tile_pool(name="outsb", bufs=6) as out_pool,
            tc.tile_pool(name="psb", bufs=2, space="PSUM") as psb_pool,
            tc.tile_pool(name="pso", bufs=3, space="PSUM") as pso_pool,
        ):
            # (S, D, X) -> sbuf [128, S//2, X]: partition p = 64*(s%2)+d,
            # free a = s//2.  In DRAM, (s, d) flattens to p-major order
            # (a p) since stride(s) = D*X and stride(d) = X.  Input loads
            # are spread over three rings so their fixed latencies overlap:
            # sync carries what the first small matmul needs (x1t a=0, W),
            # scalar/gpsimd carry the bulky x0 halves.
            wx1_sb = const_pool.tile([128, D + (S // 2) * ROWS], f32r)
            wt = wx1_sb[:, 0:D]
            x0h_sb = const_pool.tile([128, S // 2, N], f16)
            x0h_r = x0h_dram.rearrange("(a ps) d x -> (ps d) a x", ps=2)
            x0_parts = [(x0h_sb, x0h_r)]
            if passes >= 3:
                x0l_sb = const_pool.tile([128, S // 2, N], f16)
                x0l_r = x0l_dram.rearrange("(a ps) d x -> (ps d) a x", ps=2)
                x0_parts.append((x0l_sb, x0l_r))

            # Split so the half feeding s=0/s=1 (W + x1t a=0) lands first
            # and the small-matmul -> bt chain starts ~1.3us earlier.
            nc.sync.dma_start(wx1_sb[:, : D + ROWS], wx1_dram[:, : D + ROWS])
            nc.sync.dma_start(wx1_sb[:, D + ROWS :], wx1_dram[:, D + ROWS :])
            for a in range(S // 2):
                for pi, (sb, r) in enumerate(x0_parts):
                    eng = nc.scalar if (a + pi) % 2 == 0 else nc.gpsimd
                    eng.dma_start(sb[:, a, :], r[:, a, :])

            for s in range(S):
                p0 = (s % 2) * D
                a = s // 2
                # Exact f32: B^T[s] = (x1[s] @ W)^T, psum[e,i] = sum_d W[d,e] x1t[d,i]
                ps_b = psb_pool.tile([D, ROWS], f32)
                nc.tensor.matmul(
                    ps_b[:],
                    wt[p0 : p0 + D, :],
                    wx1_sb[p0 : p0 + D, D + a * ROWS : D + (a + 1) * ROWS],
                    start=True,
                    stop=True,
                )
                # fp16 hi/lo split of B, computed on-device.
                bt_hi = bt_pool.tile([128, ROWS], f16)
                nc.vector.tensor_copy(bt_hi[p0 : p0 + D, :], ps_b[:])
                if passes >= 2:
                    bt_lo = bt_pool.tile([128, ROWS], f16)
                    nc.vector.tensor_sub(
                        bt_lo[p0 : p0 + D, :], ps_b[:], bt_hi[p0 : p0 + D, :]
                    )

                for it in range(IT):
                    out_sb = out_pool.tile([128, N], out_dt)
                    isl = slice(it * 128, (it + 1) * 128)
                    # Two matmul columns share a 2-bank psum tile so each
                    # copy-back moves 1024 cols (amortizes per-op overhead).
                    for jt2 in range(JT // 2):
                        ps_o = pso_pool.tile([128, 1024], f32)
                        terms = []
                        if passes >= 2:
                            terms.append((bt_lo, x0h_sb))
                        terms.append((bt_hi, x0h_sb))
                        if passes >= 3:
                            terms.append((bt_hi, x0l_sb))
                        nt = len(terms)
                        for t, (bt_t, x0_t) in enumerate(terms):
                            for h in range(2):
                                jt = jt2 * 2 + h
                                nc.tensor.matmul(
                                    ps_o[:, h * 512 : (h + 1) * 512],
                                    bt_t[p0 : p0 + D, isl],
                                    x0_t[p0 : p0 + D, a, jt * 512 : (jt + 1) * 512],
                                    start=(t == 0),
                                    stop=(t == nt - 1),
                                )
                        dst = out_sb[:, jt2 * 1024 : (jt2 + 1) * 1024]
                        # Copies split ACT/DVE. The very first block stays
                        # all-DVE to reach the first out DMA sooner.
                        first_block = s == 0 and it == 0
                        if out_dt == f32:
                            # f32 out: DMA-bound on both HWDGE rings; ACT
                            # takes one copy per block so its ring's DMA
                            # issues don't stall behind copies.
                            act_copy = jt2 == 0 and not first_block
                        else:
                            # f16 out: DMA fits on the sync ring alone, so
                            # ACT is free to take half the copies and the
                            # DVE/PE pipeline runs unthrottled.
                            act_copy = jt2 % 2 == 0 and not first_block
                        if act_copy:
                            nc.scalar.copy(dst, ps_o[:])
                        else:
                            nc.vector.tensor_copy(dst, ps_o[:])
                        if jt2 % 2 == 1:
                            # Drain each finished 2048-col half-block right
                            # away, alternating HWDGE rings for f32 out;
                            # sync-only for f16 out (ACT ring stays copy-only).
                            jh = jt2 // 2
                            if out_dt == f32:
                                eng = nc.sync if (it + jh) % 2 == 0 else nc.scalar
                            else:
                                eng = nc.sync
                            nsl = slice(jh * 2048, (jh + 1) * 2048)
                            eng.dma_start(out_dram[s, isl, nsl], out_sb[:, nsl])
    nc.compile()
    return nc


def _get_nc():
    key = (PASSES, BIG_DT, OUT_DT)
    if key not in _CACHE:
        _CACHE[key] = _build(PASSES, BIG_DT, OUT_DT)
    return _CACHE[key]


LAST_RESULTS = None


def kernel(**inputs):
    from concourse.bass_utils import run_bass_kernel_spmd

    global LAST_RESULTS

    tensor0 = np.ascontiguousarray(np.asarray(inputs["tensor0"], dtype=np.float32))
    tensor1 = np.ascontiguousarray(np.asarray(inputs["tensor1"], dtype=np.float32))
    W = np.ascontiguousarray(np.asarray(inputs["kernel"], dtype=np.float32))
    bias = float(np.asarray(inputs["bias"]))

    # Host prep: contraction dim to axis -2 for partition-major DMA, then
    # fp16 hi/lo split of x0.
    x0t = np.ascontiguousarray(tensor0.transpose(0, 2, 1))  # (S, D, N)
    x0h = x0t.astype(np.float16) if BIG_DT == "float16" else x0t
    x1t_full = tensor1.transpose(0, 2, 1)  # (S, D, N) view

    base = {"x0h": x0h}
    if PASSES >= 3:
        base["x0l"] = (x0t - x0h.astype(np.float32)).astype(np.float16)

    in_maps = []
    for c in range(N_CORES):
        m = dict(base)
        # Pack [W | x1t(a=0) | x1t(a=1)] partition-major (p = 64*(s%2)+d)
        # into one array so a single DMA feeds the small matmuls.
        x1c = x1t_full[:, :, c * ROWS : (c + 1) * ROWS]  # (S, D, ROWS)
        wx1 = np.empty((128, D + (S // 2) * ROWS), dtype=np.float32)
        wx1[0:D, 0:D] = W
        wx1[D : 2 * D, 0:D] = W
        for a in range(S // 2):
            csl = slice(D + a * ROWS, D + (a + 1) * ROWS)
            wx1[0:D, csl] = x1c[2 * a]
            wx1[D : 2 * D, csl] = x1c[2 * a + 1]
        m["wx1"] = wx1
        in_maps.append(m)

    nc = _get_nc()
    res = run_bass_kernel_spmd(nc, in_maps, list(range(N_CORES)))
    LAST_RESULTS = res

    out_full = np.empty((S, N, N), dtype=np.float32)
    for c in range(N_CORES):
        out_full[:, c * ROWS : (c + 1) * ROWS, :] = res.results[c]["out"].astype(
            np.float32, copy=False
        )

    if bias != 0.0:
        out_full += np.float32(bias)

    return np.broadcast_to(out_full[None], (BATCH, S, N, N))
